# revision 17
# baseline (speedup 1.0000x reference)
"""Trainium2 Bass kernel for the ConstraintLoss problem (8-core SPMD).

Contract: kernel(**inputs) takes the FULL unsharded inputs (numpy or jax
arrays, keyed as in setup_inputs()) and returns the full output — the
8-tuple of scalar losses stacked into a float32 array of shape (8,):
  [L_total, L_recon, L_rule, L_attn, L_attn_gat, L_attn_rule, L_reg,
   num_violations]

Sharding strategy (host side = structure prep + shard/unshard only; the
floating-point reduction math runs on the 8 NeuronCores):
  * Cars (180000) are sharded by ordinal range across the 8 cores
    (22528 rows/core, padded); the packed score vector follows the split.
  * The edge-wise segment-max over source-node segments becomes a dense
    per-car reduction: the host bins each car's rule-edge alphas (edges
    whose dst is a light/stop) into a padded [rows, K=16] bf16 table
    (payload (alpha+1)/2, 0 = empty slot). Each core max-reduces its
    shard with a TT-max tree on the vector engine — the distributed
    segment-max from the sharding hint, with the node space sharded so
    no cross-core combine is needed.
  * param0/param1 are flattened, pre-scaled by 64, converted to fp8-e4m3
    and split per core; each core's block is split between the scalar
    engine (Square activation with accum_out) and the vector engine
    (fused scalar_tensor_tensor), both one-pass sum-of-squares; the host
    divides the sums by 64^2.
  * Each core emits a [128,16] tile of per-partition partial sums; the
    host folds the partition rows, adds the 8 per-core vectors, and
    applies the final scalar formula.
"""

import numpy as np
from contextlib import ExitStack

import ml_dtypes

import concourse.bacc as bacc
import concourse.mybir as mybir
import concourse.tile as tile
from concourse.bass_utils import run_bass_kernel_spmd

F32 = mybir.dt.float32
BF16 = mybir.dt.bfloat16
FP8 = mybir.dt.float8e4
ALU = mybir.AluOpType
ACTF = mybir.ActivationFunctionType

# Problem constants (hardcoded per the task contract).
N_CAR = 180000
N = 200000
NCORES = 8

G = 176                   # row groups per partition
RPC = 128 * G             # 22528 rows (car ordinals) per core
ROWS = RPC * NCORES       # 180224 padded rows
NPAD = ROWS - N_CAR       # 224 pad rows (all on core 7)
K = 16                    # padded rule-edge slots per car (max degree 15)

PTOT = 32768              # param elems per partition per core (2*4096^2/8/128)
A_TILES = [2048, 4832, 4832, 4832, 4832]   # ACT (fp8) tile free-dims
D_TILES = [2848, 2848, 2848, 2848]         # DVE (fp8) tile free-dims
PA = sum(A_TILES)
PD = sum(D_TILES)
assert PA + PD == PTOT, (PA, PD)
FP8_SCALE = 64.0          # host pre-scale for the fp8 streams
FP8_CLIP = 240.0          # ml_dtypes.float8_e4m3 max finite

LAMBDA_RECON, LAMBDA_RULE, LAMBDA_ATTN, LAMBDA_REG = 1.0, 0.5, 0.3, 1e-4
W_ATTN_GAT, W_ATTN_RULE = 0.5, 0.5

# BCE pad rows carry ms=0.5, rsb=0.5 -> d=ln1-ln2=0 so s1 gets nothing,
# s2 gets exactly ln(0.5) per pad row (subtracted on the host), and the
# strict rsb>0.5 violation test stays false.
_PAD_MS = 0.5

_NC = None


def _build_nc():
    """Build + compile the per-core Bass program (SPMD, identical on all
    cores; only the input shards differ)."""
    nc = bacc.Bacc("TRN2", target_bir_lowering=False, debug=False,
                   enable_asserts=True, num_devices=NCORES)

    pa = nc.dram_tensor("pa", [128, PA], FP8, kind="ExternalInput")
    pd = nc.dram_tensor("pd", [128, PD], FP8, kind="ExternalInput")
    ptab = nc.dram_tensor("ptab", [128, G * K], BF16, kind="ExternalInput")
    sc = nc.dram_tensor("sc", [128, 3 * G], F32, kind="ExternalInput")
    out = nc.dram_tensor("partials", [128, 16], F32, kind="ExternalOutput")

    a_off = np.cumsum([0] + A_TILES)
    d_off = np.cumsum([0] + D_TILES)

    with ExitStack() as ctx:
        tc = ctx.enter_context(tile.TileContext(nc))
        sp = ctx.enter_context(tc.tile_pool(name="small", bufs=1))
        pp = ctx.enter_context(tc.tile_pool(name="params", bufs=3))
        scrp = ctx.enter_context(tc.tile_pool(name="scratch", bufs=2))

        # ---- input DMAs spread over three rings so trigger serialization
        # overlaps and the compute engines' sequencers stay free: SP ring
        # carries sc + ACT tiles, the ACT ring only the table (one cheap
        # trigger before ACT's compute), SWDGE (gpsimd, idle) the DVE tiles.
        t_sc = sp.tile([128, 3 * G], F32)
        t_tab = sp.tile([128, G * K], BF16)
        pa_tiles = [None] * len(A_TILES)
        pd_tiles = [None] * len(D_TILES)

        nc.sync.dma_start(t_sc[:], sc.ap())
        nc.scalar.dma_start(t_tab[:], ptab.ap())
        for t in range(len(A_TILES)):
            tl = pp.tile([128, A_TILES[t]], FP8, tag=f"pa{t}")
            nc.sync.dma_start(tl[:], pa.ap()[:, int(a_off[t]):int(a_off[t + 1])])
            pa_tiles[t] = tl
        for t in range(len(D_TILES)):
            tl = pp.tile([128, D_TILES[t]], FP8, tag=f"pd{t}")
            nc.gpsimd.dma_start(tl[:], pd.ap()[:, int(d_off[t]):int(d_off[t + 1])])
            pd_tiles[t] = tl

        t_ms = t_sc[:, 0 * G:1 * G]
        t_rsb = t_sc[:, 1 * G:2 * G]
        t_bet = t_sc[:, 2 * G:3 * G]

        # parts: one column per partial quantity, written in place by
        # accum_out of the producing op; host folds the partition rows.
        #  0:s1  1:s2  2:srule  3:nv  4:sar  5:scnt  6:sgat  7:spa  8:spd
        parts = sp.tile([128, 16], F32)
        nc.vector.memset(parts[:], 0.0)

        # ---- scalar-engine ops: ln/bsq then the fp8 squares ----
        ln1 = sp.tile([128, G], F32)
        nc.scalar.activation(ln1[:], t_ms, ACTF.Ln)
        ln2 = sp.tile([128, G], F32)
        nc.scalar.activation(ln2[:], t_ms, ACTF.Ln, scale=-1.0, bias=1.0,
                             accum_out=parts[:, 1:2])
        bsq = sp.tile([128, G], F32)
        nc.scalar.activation(bsq[:], t_bet, ACTF.Square, scale=-1.0, bias=1.0)
        acca = sp.tile([128, len(A_TILES)], F32)
        for t in range(len(A_TILES)):
            sq = scrp.tile([128, A_TILES[t]], BF16, tag="sqa")
            nc.scalar.activation(sq[:], pa_tiles[t][:], ACTF.Square,
                                 accum_out=acca[:, t:t + 1])

        # ---- vector-engine ops ----
        # GAT rowmax via TT-max tree over the K=16 slots (bf16, 2x packed)
        tabv = t_tab[:].rearrange("p (k g) -> p k g", k=K)
        m8 = sp.tile([128, 8 * G], BF16)
        m8v = m8[:].rearrange("p (k g) -> p k g", k=8)
        nc.vector.tensor_tensor(m8v, tabv[:, 0:8, :], tabv[:, 8:16, :], ALU.max)
        m4 = sp.tile([128, 4 * G], BF16)
        m4v = m4[:].rearrange("p (k g) -> p k g", k=4)
        nc.vector.tensor_tensor(m4v, m8v[:, 0:4, :], m8v[:, 4:8, :], ALU.max)
        m2 = sp.tile([128, 2 * G], BF16)
        m2v = m2[:].rearrange("p (k g) -> p k g", k=2)
        nc.vector.tensor_tensor(m2v, m4v[:, 0:2, :], m4v[:, 2:4, :], ALU.max)
        rowmax = sp.tile([128, G], F32)
        nc.vector.tensor_tensor(rowmax[:], m2v[:, 0, :], m2v[:, 1, :], ALU.max)

        viol = sp.tile([128, G], F32)   # rsb>0.5 (strict: pad rows stay 0)
        nc.vector.tensor_scalar(viol[:], t_rsb, 0.5, 0.0,
                                ALU.is_gt, ALU.add, accum_out=parts[:, 3:4])
        has = sp.tile([128, G], F32)    # car has >=1 rule edge
        nc.vector.tensor_scalar(has[:], rowmax[:], 0.5, None, ALU.is_ge)
        dd = sp.tile([128, G], F32)     # 1 - payload == (1 - max_alpha)/2
        nc.vector.tensor_scalar(dd[:], rowmax[:], -1.0, 1.0, ALU.mult, ALU.add)

        gscr = sp.tile([128, G], F32)   # throwaway elementwise outputs
        d12 = sp.tile([128, G], F32)
        nc.vector.tensor_tensor(d12[:], ln1[:], ln2[:], ALU.subtract)
        nc.vector.scalar_tensor_tensor(gscr[:], t_rsb, 1.0, d12[:],
                                       ALU.mult, ALU.mult,
                                       accum_out=parts[:, 0:1])
        diff = sp.tile([128, G], F32)
        nc.vector.tensor_tensor(diff[:], t_ms, t_rsb, ALU.subtract)
        nc.vector.scalar_tensor_tensor(gscr[:], diff[:], 1.0, diff[:],
                                       ALU.mult, ALU.mult,
                                       accum_out=parts[:, 2:3])
        nc.vector.scalar_tensor_tensor(gscr[:], viol[:], 1.0, bsq[:],
                                       ALU.mult, ALU.mult,
                                       accum_out=parts[:, 4:5])
        valid = sp.tile([128, G], F32)
        nc.vector.scalar_tensor_tensor(valid[:], has[:], 1.0, viol[:],
                                       ALU.mult, ALU.mult,
                                       accum_out=parts[:, 5:6])
        vd = sp.tile([128, G], F32)     # valid*dd; then sum(vd*dd)=sum valid*dd^2
        nc.vector.scalar_tensor_tensor(vd[:], valid[:], 1.0, dd[:],
                                       ALU.mult, ALU.mult)
        nc.vector.scalar_tensor_tensor(gscr[:], vd[:], 1.0, dd[:],
                                       ALU.mult, ALU.mult,
                                       accum_out=parts[:, 6:7])

        # DVE param stream (fp8 STT sum-of-squares)
        accd = sp.tile([128, len(D_TILES)], F32)
        for t in range(len(D_TILES)):
            sq = scrp.tile([128, D_TILES[t]], BF16, tag="sqd")
            nc.vector.scalar_tensor_tensor(sq[:], pd_tiles[t][:], 1.0,
                                           pd_tiles[t][:], ALU.mult, ALU.mult,
                                           accum_out=accd[:, t:t + 1])

        nc.vector.tensor_reduce(parts[:, 7:8], acca[:],
                                mybir.AxisListType.X, ALU.add)
        nc.vector.tensor_reduce(parts[:, 8:9], accd[:],
                                mybir.AxisListType.X, ALU.add)

        # ---- ship per-partition partials; host folds the 128 rows ----
        nc.sync.dma_start(out.ap(), parts[:])

    nc.compile()
    return nc


def _get_nc():
    global _NC
    if _NC is None:
        _NC = _build_nc()
    return _NC


def prep_in_maps(inputs):
    """Host-side structure prep + sharding. Returns per-core input dicts."""
    ms = np.asarray(inputs["model_scores"], np.float32)
    rs = np.asarray(inputs["rule_scores"], np.float32)
    alpha = np.asarray(inputs["alpha_gat"], np.float32)
    beta = np.asarray(inputs["beta_rule"], np.float32)
    ei = np.asarray(inputs["edge_index"])
    et = np.asarray(inputs["entity_types"])
    p0 = np.asarray(inputs["param0"], np.float32)
    p1 = np.asarray(inputs["param1"], np.float32)

    src = ei[0].astype(np.int64, copy=False)
    dst = ei[1].astype(np.int64, copy=False)

    # rule edges: dst is a light (1) or stop line (2)
    rule_node = (et == 1) | (et == 2)
    sel = rule_node[dst]
    src_r = src[sel]
    a_r = alpha[sel]

    # group rule-edge alphas by source node (CSR-style)
    order = np.argsort(src_r, kind="stable")
    ssrc = src_r[order]
    sa = a_r[order]
    counts = np.bincount(ssrc, minlength=N)
    starts = np.zeros_like(counts)
    starts[1:] = np.cumsum(counts[:-1])

    # car ordinal -> node id (reference: nonzero(et==0, size=N_CAR), fill 0)
    car_ids = np.nonzero(et == 0)[0]
    if car_ids.size >= N_CAR:
        car_ids = car_ids[:N_CAR]
    else:
        car_ids = np.concatenate(
            [car_ids, np.zeros(N_CAR - car_ids.size, car_ids.dtype)])

    # padded [ROWS, K] table of (alpha+1)/2, one row per car ordinal
    cnt_full = counts[car_ids]
    cnt_ord = np.minimum(cnt_full, K)
    tot = int(cnt_ord.sum())
    row_idx = np.repeat(np.arange(N_CAR, dtype=np.int64), cnt_ord)
    cum = np.cumsum(cnt_ord) - cnt_ord
    within = np.arange(tot, dtype=np.int64) - np.repeat(cum, cnt_ord)
    srcpos = np.repeat(starts[car_ids], cnt_ord) + within
    ptab = np.zeros(ROWS * K, np.float32)
    ptab[row_idx * K + within] = (sa[srcpos] + np.float32(1.0)) * np.float32(0.5)
    # overflow fold (degree > K; not hit for the graded distribution)
    for i in np.nonzero(cnt_full > K)[0]:
        node = car_ids[i]
        extra = (sa[starts[node] + K:starts[node] + cnt_full[i]]
                 + np.float32(1.0)) * np.float32(0.5)
        ptab[i * K + K - 1] = max(ptab[i * K + K - 1], extra.max())
    ptab = ptab.reshape(ROWS, K)

    # padded score rows
    def pad(v, fill):
        o = np.full(ROWS, fill, np.float32)
        o[:N_CAR] = v
        return o

    ms_p = pad(ms, _PAD_MS)
    rsb_p = pad(rs, _PAD_MS)   # pad: ln(0.5) into s2 only; viol strict ✓
    bet_p = pad(beta, 1.0)

    # params: flatten, split per core, scale+convert to fp8
    pflat = np.concatenate([p0.reshape(-1), p1.reshape(-1)])
    pcore = pflat.reshape(NCORES, 128, PTOT)
    np_fp8 = mybir.dt.np(FP8)

    in_maps = []
    for c in range(NCORES):
        r0, r1 = c * RPC, (c + 1) * RPC
        scp = np.stack([ms_p[r0:r1].reshape(128, G),
                        rsb_p[r0:r1].reshape(128, G),
                        bet_p[r0:r1].reshape(128, G)], axis=1)
        tabc = (ptab[r0:r1].reshape(128, G, K).transpose(0, 2, 1)
                .astype(ml_dtypes.bfloat16))
        pblk = np.clip(pcore[c] * np.float32(FP8_SCALE),
                       -FP8_CLIP, FP8_CLIP).astype(np_fp8)
        in_maps.append({
            "pa": np.ascontiguousarray(pblk[:, :PA]),
            "pd": np.ascontiguousarray(pblk[:, PA:]),
            "ptab": np.ascontiguousarray(tabc).reshape(128, K * G),
            "sc": np.ascontiguousarray(scp).reshape(128, 3 * G),
        })
    return in_maps


def combine_partials(partials_per_core):
    """Host unshard: fold partition rows, add the 8 partial vectors,
    apply the scalar formula."""
    s = np.zeros(16, np.float64)
    for p in partials_per_core:
        s += np.asarray(p, np.float64).reshape(128, 16).sum(axis=0)
    s1, s2, s_rule, nv, s_ar, s_cnt, s_gat, s_pa, s_pd = s[:9]
    s_bce = s1 + s2 - NPAD * np.log(0.5)  # remove constant pad contribution

    L_recon = -s_bce / N_CAR
    L_rule = s_rule / N_CAR
    any_viol = nv > 0
    # payload = (alpha+1)/2 so (1-payload)^2 = (1-alpha)^2/4 -> x4 here
    L_attn_gat = (4.0 * s_gat / max(s_cnt, 1.0)) if (any_viol and s_cnt > 0) else 0.0
    L_attn_rule = (s_ar / max(nv, 1.0)) if any_viol else 0.0
    L_attn = W_ATTN_GAT * L_attn_gat + W_ATTN_RULE * L_attn_rule
    L_reg = (s_pa + s_pd) / (FP8_SCALE * FP8_SCALE)
    L_total = (LAMBDA_RECON * L_recon + LAMBDA_RULE * L_rule
               + LAMBDA_ATTN * L_attn + LAMBDA_REG * L_reg)
    return np.array([L_total, L_recon, L_rule, L_attn, L_attn_gat,
                     L_attn_rule, L_reg, nv], np.float32)


def kernel(**inputs):
    nc = _get_nc()
    in_maps = prep_in_maps(inputs)
    res = run_bass_kernel_spmd(nc, in_maps, list(range(NCORES)))
    return combine_partials([r["partials"] for r in res.results])


# revision 19
# speedup vs baseline: 1.0629x; 1.0629x over previous
"""Trainium2 Bass kernel for the ConstraintLoss problem (8-core SPMD).

Contract: kernel(**inputs) takes the FULL unsharded inputs (numpy or jax
arrays, keyed as in setup_inputs()) and returns the full output — the
8-tuple of scalar losses stacked into a float32 array of shape (8,):
  [L_total, L_recon, L_rule, L_attn, L_attn_gat, L_attn_rule, L_reg,
   num_violations]

Sharding strategy (host side = structure prep + shard/unshard only; the
floating-point reduction math runs on the 8 NeuronCores):
  * Cars (180000) are sharded by ordinal range across the 8 cores
    (22528 rows/core, padded); the packed score vector follows the split.
  * The edge-wise segment-max over source-node segments becomes a dense
    per-car reduction: the host bins each car's rule-edge alphas (edges
    whose dst is a light/stop) into a padded [rows, K=16] bf16 table
    (payload (alpha+1)/2, 0 = empty slot). Each core max-reduces its
    shard with a TT-max tree on the vector engine — the distributed
    segment-max from the sharding hint, with the node space sharded so
    no cross-core combine is needed.
  * param0/param1 are flattened, pre-scaled by 64, converted to fp8-e4m3
    and split per core; each core's block is split between the scalar
    engine (Square activation with accum_out) and the vector engine
    (fused scalar_tensor_tensor), both one-pass sum-of-squares; the host
    divides the sums by 64^2.
  * Each core emits a [128,16] tile of per-partition partial sums; the
    host folds the partition rows, adds the 8 per-core vectors, and
    applies the final scalar formula.
"""

import numpy as np
from contextlib import ExitStack

import ml_dtypes

import concourse.bacc as bacc
import concourse.mybir as mybir
import concourse.tile as tile
from concourse.bass_utils import run_bass_kernel_spmd

F32 = mybir.dt.float32
BF16 = mybir.dt.bfloat16
FP8 = mybir.dt.float8e4
ALU = mybir.AluOpType
ACTF = mybir.ActivationFunctionType

# Problem constants (hardcoded per the task contract).
N_CAR = 180000
N = 200000
NCORES = 8

G = 176                   # row groups per partition
RPC = 128 * G             # 22528 rows (car ordinals) per core
ROWS = RPC * NCORES       # 180224 padded rows
NPAD = ROWS - N_CAR       # 224 pad rows (all on core 7)
K = 16                    # padded rule-edge slots per car (max degree 15)

PTOT = 32768              # param elems per partition per core (2*4096^2/8/128)
A_TILES = [2048, 4832, 4832, 4832, 4832]   # ACT (fp8) tile free-dims
D_TILES = [2848, 2848, 2848, 2848]         # DVE (fp8) tile free-dims
PA = sum(A_TILES)
PD = sum(D_TILES)
assert PA + PD == PTOT, (PA, PD)
FP8_SCALE = 64.0          # host pre-scale for the fp8 streams
FP8_CLIP = 240.0          # ml_dtypes.float8_e4m3 max finite

LAMBDA_RECON, LAMBDA_RULE, LAMBDA_ATTN, LAMBDA_REG = 1.0, 0.5, 0.3, 1e-4
W_ATTN_GAT, W_ATTN_RULE = 0.5, 0.5

# BCE pad rows carry ms=0.5, rsb=0.5 -> d=ln1-ln2=0 so s1 gets nothing,
# s2 gets exactly ln(0.5) per pad row (subtracted on the host), and the
# strict rsb>0.5 violation test stays false.
_PAD_MS = 0.5

_NC = None


def _build_nc():
    """Build + compile the per-core Bass program (SPMD, identical on all
    cores; only the input shards differ)."""
    nc = bacc.Bacc("TRN2", target_bir_lowering=False, debug=False,
                   enable_asserts=True, num_devices=NCORES)

    pa = nc.dram_tensor("pa", [128, PA], FP8, kind="ExternalInput")
    pd = nc.dram_tensor("pd", [128, PD], FP8, kind="ExternalInput")
    ptab = nc.dram_tensor("ptab", [128, G * K], BF16, kind="ExternalInput")
    sc = nc.dram_tensor("sc", [128, 3 * G], F32, kind="ExternalInput")
    out = nc.dram_tensor("partials", [128, 16], F32, kind="ExternalOutput")

    a_off = np.cumsum([0] + A_TILES)
    d_off = np.cumsum([0] + D_TILES)

    with ExitStack() as ctx:
        tc = ctx.enter_context(tile.TileContext(nc))
        sp = ctx.enter_context(tc.tile_pool(name="small", bufs=1))
        pp = ctx.enter_context(tc.tile_pool(name="params", bufs=3))
        scrp = ctx.enter_context(tc.tile_pool(name="scratch", bufs=2))

        # ---- input DMAs spread over three rings so trigger serialization
        # overlaps and the compute engines' sequencers stay free: SP ring
        # carries sc + ACT tiles, the ACT ring only the table (one cheap
        # trigger before ACT's compute), SWDGE (gpsimd, idle) the DVE tiles.
        t_sc = sp.tile([128, 3 * G], F32)
        t_tab = sp.tile([128, G * K], BF16)
        pa_tiles = [None] * len(A_TILES)
        pd_tiles = [None] * len(D_TILES)

        nc.sync.dma_start(t_sc[:], sc.ap())
        nc.scalar.dma_start(t_tab[:], ptab.ap())
        for t in range(len(A_TILES)):
            tl = pp.tile([128, A_TILES[t]], FP8, tag=f"pa{t}")
            nc.sync.dma_start(tl[:], pa.ap()[:, int(a_off[t]):int(a_off[t + 1])])
            pa_tiles[t] = tl
        for t in range(len(D_TILES)):
            tl = pp.tile([128, D_TILES[t]], FP8, tag=f"pd{t}")
            nc.sync.dma_start(tl[:], pd.ap()[:, int(d_off[t]):int(d_off[t + 1])])
            pd_tiles[t] = tl

        t_ms = t_sc[:, 0 * G:1 * G]
        t_rsb = t_sc[:, 1 * G:2 * G]
        t_bet = t_sc[:, 2 * G:3 * G]

        # parts: one column per partial quantity, written in place by
        # accum_out of the producing op; host folds the partition rows.
        #  0:s1  1:s2  2:srule  3:nv  4:sar  5:scnt  6:sgat  7:spa  8:spd
        parts = sp.tile([128, 16], F32)
        nc.vector.memset(parts[:], 0.0)

        # ---- scalar-engine ops: ln/bsq then the fp8 squares ----
        ln1 = sp.tile([128, G], F32)
        nc.scalar.activation(ln1[:], t_ms, ACTF.Ln)
        ln2 = sp.tile([128, G], F32)
        nc.scalar.activation(ln2[:], t_ms, ACTF.Ln, scale=-1.0, bias=1.0,
                             accum_out=parts[:, 1:2])
        bsq = sp.tile([128, G], F32)
        nc.scalar.activation(bsq[:], t_bet, ACTF.Square, scale=-1.0, bias=1.0)
        acca = sp.tile([128, len(A_TILES)], F32)
        for t in range(len(A_TILES)):
            sq = scrp.tile([128, A_TILES[t]], BF16, tag="sqa")
            nc.scalar.activation(sq[:], pa_tiles[t][:], ACTF.Square,
                                 accum_out=acca[:, t:t + 1])

        # ---- vector-engine ops ----
        # GAT rowmax via TT-max tree over the K=16 slots (bf16, 2x packed)
        tabv = t_tab[:].rearrange("p (k g) -> p k g", k=K)
        m8 = sp.tile([128, 8 * G], BF16)
        m8v = m8[:].rearrange("p (k g) -> p k g", k=8)
        nc.vector.tensor_tensor(m8v, tabv[:, 0:8, :], tabv[:, 8:16, :], ALU.max)
        m4 = sp.tile([128, 4 * G], BF16)
        m4v = m4[:].rearrange("p (k g) -> p k g", k=4)
        nc.vector.tensor_tensor(m4v, m8v[:, 0:4, :], m8v[:, 4:8, :], ALU.max)
        m2 = sp.tile([128, 2 * G], BF16)
        m2v = m2[:].rearrange("p (k g) -> p k g", k=2)
        nc.vector.tensor_tensor(m2v, m4v[:, 0:2, :], m4v[:, 2:4, :], ALU.max)
        rowmax = sp.tile([128, G], F32)
        nc.vector.tensor_tensor(rowmax[:], m2v[:, 0, :], m2v[:, 1, :], ALU.max)

        viol = sp.tile([128, G], F32)   # rsb>0.5 (strict: pad rows stay 0)
        nc.vector.tensor_scalar(viol[:], t_rsb, 0.5, 0.0,
                                ALU.is_gt, ALU.add, accum_out=parts[:, 3:4])
        has = sp.tile([128, G], F32)    # car has >=1 rule edge
        nc.vector.tensor_scalar(has[:], rowmax[:], 0.5, None, ALU.is_ge)
        dd = sp.tile([128, G], F32)     # 1 - payload == (1 - max_alpha)/2
        nc.vector.tensor_scalar(dd[:], rowmax[:], -1.0, 1.0, ALU.mult, ALU.add)

        gscr = sp.tile([128, G], F32)   # throwaway elementwise outputs
        d12 = sp.tile([128, G], F32)
        nc.vector.tensor_tensor(d12[:], ln1[:], ln2[:], ALU.subtract)
        nc.vector.scalar_tensor_tensor(gscr[:], t_rsb, 1.0, d12[:],
                                       ALU.mult, ALU.mult,
                                       accum_out=parts[:, 0:1])
        diff = sp.tile([128, G], F32)
        nc.vector.tensor_tensor(diff[:], t_ms, t_rsb, ALU.subtract)
        nc.vector.scalar_tensor_tensor(gscr[:], diff[:], 1.0, diff[:],
                                       ALU.mult, ALU.mult,
                                       accum_out=parts[:, 2:3])
        nc.vector.scalar_tensor_tensor(gscr[:], viol[:], 1.0, bsq[:],
                                       ALU.mult, ALU.mult,
                                       accum_out=parts[:, 4:5])
        valid = sp.tile([128, G], F32)
        nc.vector.scalar_tensor_tensor(valid[:], has[:], 1.0, viol[:],
                                       ALU.mult, ALU.mult,
                                       accum_out=parts[:, 5:6])
        vd = sp.tile([128, G], F32)     # valid*dd; then sum(vd*dd)=sum valid*dd^2
        nc.vector.scalar_tensor_tensor(vd[:], valid[:], 1.0, dd[:],
                                       ALU.mult, ALU.mult)
        nc.vector.scalar_tensor_tensor(gscr[:], vd[:], 1.0, dd[:],
                                       ALU.mult, ALU.mult,
                                       accum_out=parts[:, 6:7])

        # DVE param stream (fp8 STT sum-of-squares)
        accd = sp.tile([128, len(D_TILES)], F32)
        for t in range(len(D_TILES)):
            sq = scrp.tile([128, D_TILES[t]], BF16, tag="sqd")
            nc.vector.scalar_tensor_tensor(sq[:], pd_tiles[t][:], 1.0,
                                           pd_tiles[t][:], ALU.mult, ALU.mult,
                                           accum_out=accd[:, t:t + 1])

        nc.vector.tensor_reduce(parts[:, 7:8], acca[:],
                                mybir.AxisListType.X, ALU.add)
        nc.vector.tensor_reduce(parts[:, 8:9], accd[:],
                                mybir.AxisListType.X, ALU.add)

        # ---- ship per-partition partials; host folds the 128 rows ----
        nc.sync.dma_start(out.ap(), parts[:])

    nc.compile()
    return nc


def _get_nc():
    global _NC
    if _NC is None:
        _NC = _build_nc()
    return _NC


def prep_in_maps(inputs):
    """Host-side structure prep + sharding. Returns per-core input dicts."""
    ms = np.asarray(inputs["model_scores"], np.float32)
    rs = np.asarray(inputs["rule_scores"], np.float32)
    alpha = np.asarray(inputs["alpha_gat"], np.float32)
    beta = np.asarray(inputs["beta_rule"], np.float32)
    ei = np.asarray(inputs["edge_index"])
    et = np.asarray(inputs["entity_types"])
    p0 = np.asarray(inputs["param0"], np.float32)
    p1 = np.asarray(inputs["param1"], np.float32)

    src = ei[0].astype(np.int64, copy=False)
    dst = ei[1].astype(np.int64, copy=False)

    # rule edges: dst is a light (1) or stop line (2)
    rule_node = (et == 1) | (et == 2)
    sel = rule_node[dst]
    src_r = src[sel]
    a_r = alpha[sel]

    # group rule-edge alphas by source node (CSR-style)
    order = np.argsort(src_r, kind="stable")
    ssrc = src_r[order]
    sa = a_r[order]
    counts = np.bincount(ssrc, minlength=N)
    starts = np.zeros_like(counts)
    starts[1:] = np.cumsum(counts[:-1])

    # car ordinal -> node id (reference: nonzero(et==0, size=N_CAR), fill 0)
    car_ids = np.nonzero(et == 0)[0]
    if car_ids.size >= N_CAR:
        car_ids = car_ids[:N_CAR]
    else:
        car_ids = np.concatenate(
            [car_ids, np.zeros(N_CAR - car_ids.size, car_ids.dtype)])

    # padded [ROWS, K] table of (alpha+1)/2, one row per car ordinal
    cnt_full = counts[car_ids]
    cnt_ord = np.minimum(cnt_full, K)
    tot = int(cnt_ord.sum())
    row_idx = np.repeat(np.arange(N_CAR, dtype=np.int64), cnt_ord)
    cum = np.cumsum(cnt_ord) - cnt_ord
    within = np.arange(tot, dtype=np.int64) - np.repeat(cum, cnt_ord)
    srcpos = np.repeat(starts[car_ids], cnt_ord) + within
    ptab = np.zeros(ROWS * K, np.float32)
    ptab[row_idx * K + within] = (sa[srcpos] + np.float32(1.0)) * np.float32(0.5)
    # overflow fold (degree > K; not hit for the graded distribution)
    for i in np.nonzero(cnt_full > K)[0]:
        node = car_ids[i]
        extra = (sa[starts[node] + K:starts[node] + cnt_full[i]]
                 + np.float32(1.0)) * np.float32(0.5)
        ptab[i * K + K - 1] = max(ptab[i * K + K - 1], extra.max())
    ptab = ptab.reshape(ROWS, K)

    # padded score rows
    def pad(v, fill):
        o = np.full(ROWS, fill, np.float32)
        o[:N_CAR] = v
        return o

    ms_p = pad(ms, _PAD_MS)
    rsb_p = pad(rs, _PAD_MS)   # pad: ln(0.5) into s2 only; viol strict ✓
    bet_p = pad(beta, 1.0)

    # params: flatten, split per core, scale+convert to fp8
    pflat = np.concatenate([p0.reshape(-1), p1.reshape(-1)])
    pcore = pflat.reshape(NCORES, 128, PTOT)
    np_fp8 = mybir.dt.np(FP8)

    in_maps = []
    for c in range(NCORES):
        r0, r1 = c * RPC, (c + 1) * RPC
        scp = np.stack([ms_p[r0:r1].reshape(128, G),
                        rsb_p[r0:r1].reshape(128, G),
                        bet_p[r0:r1].reshape(128, G)], axis=1)
        tabc = (ptab[r0:r1].reshape(128, G, K).transpose(0, 2, 1)
                .astype(ml_dtypes.bfloat16))
        pblk = np.clip(pcore[c] * np.float32(FP8_SCALE),
                       -FP8_CLIP, FP8_CLIP).astype(np_fp8)
        in_maps.append({
            "pa": np.ascontiguousarray(pblk[:, :PA]),
            "pd": np.ascontiguousarray(pblk[:, PA:]),
            "ptab": np.ascontiguousarray(tabc).reshape(128, K * G),
            "sc": np.ascontiguousarray(scp).reshape(128, 3 * G),
        })
    return in_maps


def combine_partials(partials_per_core):
    """Host unshard: fold partition rows, add the 8 partial vectors,
    apply the scalar formula."""
    s = np.zeros(16, np.float64)
    for p in partials_per_core:
        s += np.asarray(p, np.float64).reshape(128, 16).sum(axis=0)
    s1, s2, s_rule, nv, s_ar, s_cnt, s_gat, s_pa, s_pd = s[:9]
    s_bce = s1 + s2 - NPAD * np.log(0.5)  # remove constant pad contribution

    L_recon = -s_bce / N_CAR
    L_rule = s_rule / N_CAR
    any_viol = nv > 0
    # payload = (alpha+1)/2 so (1-payload)^2 = (1-alpha)^2/4 -> x4 here
    L_attn_gat = (4.0 * s_gat / max(s_cnt, 1.0)) if (any_viol and s_cnt > 0) else 0.0
    L_attn_rule = (s_ar / max(nv, 1.0)) if any_viol else 0.0
    L_attn = W_ATTN_GAT * L_attn_gat + W_ATTN_RULE * L_attn_rule
    L_reg = (s_pa + s_pd) / (FP8_SCALE * FP8_SCALE)
    L_total = (LAMBDA_RECON * L_recon + LAMBDA_RULE * L_rule
               + LAMBDA_ATTN * L_attn + LAMBDA_REG * L_reg)
    return np.array([L_total, L_recon, L_rule, L_attn, L_attn_gat,
                     L_attn_rule, L_reg, nv], np.float32)


def kernel(**inputs):
    nc = _get_nc()
    in_maps = prep_in_maps(inputs)
    res = run_bass_kernel_spmd(nc, in_maps, list(range(NCORES)))
    return combine_partials([r["partials"] for r in res.results])


# revision 21
# speedup vs baseline: 1.1105x; 1.0447x over previous
"""Trainium2 Bass kernel for the ConstraintLoss problem (8-core SPMD).

Contract: kernel(**inputs) takes the FULL unsharded inputs (numpy or jax
arrays, keyed as in setup_inputs()) and returns the full output — the
8-tuple of scalar losses stacked into a float32 array of shape (8,):
  [L_total, L_recon, L_rule, L_attn, L_attn_gat, L_attn_rule, L_reg,
   num_violations]

Sharding strategy (host side = structure prep + shard/unshard only; the
floating-point reduction math runs on the 8 NeuronCores):
  * Cars (180000) are sharded by ordinal range across the 8 cores
    (22528 rows/core, padded); the packed score vector follows the split.
  * The edge-wise segment-max over source-node segments becomes a dense
    per-car reduction: the host bins each car's rule-edge alphas (edges
    whose dst is a light/stop) into a padded [rows, K=16] bf16 table
    (payload (alpha+1)/2, 0 = empty slot). Each core max-reduces its
    shard with a TT-max tree on the vector engine — the distributed
    segment-max from the sharding hint, with the node space sharded so
    no cross-core combine is needed.
  * param0/param1 are flattened, pre-scaled by 64, converted to fp8-e4m3
    and split per core; each core's block is split between the scalar
    engine (Square activation with accum_out) and the vector engine
    (fused scalar_tensor_tensor), both one-pass sum-of-squares; the host
    divides the sums by 64^2.
  * Each core emits a [128,16] tile of per-partition partial sums; the
    host folds the partition rows, adds the 8 per-core vectors, and
    applies the final scalar formula.
"""

import numpy as np
from contextlib import ExitStack

import ml_dtypes

import concourse.bacc as bacc
import concourse.mybir as mybir
import concourse.tile as tile
from concourse.bass_utils import run_bass_kernel_spmd

F32 = mybir.dt.float32
BF16 = mybir.dt.bfloat16
FP8 = mybir.dt.float8e4
ALU = mybir.AluOpType
ACTF = mybir.ActivationFunctionType

# Problem constants (hardcoded per the task contract).
N_CAR = 180000
N = 200000
NCORES = 8

G = 176                   # row groups per partition
RPC = 128 * G             # 22528 rows (car ordinals) per core
ROWS = RPC * NCORES       # 180224 padded rows
NPAD = ROWS - N_CAR       # 224 pad rows (all on core 7)
K = 16                    # padded rule-edge slots per car (max degree 15)

PTOT = 32768              # param elems per partition per core (2*4096^2/8/128)
A_TILES = [2048, 8832, 8832]               # ACT (fp8) tile free-dims
D_TILES = [4352, 4352, 4352]               # DVE (fp8) tile free-dims
PA = sum(A_TILES)
PD = sum(D_TILES)
assert PA + PD == PTOT, (PA, PD)
FP8_SCALE = 64.0          # host pre-scale for the fp8 streams
FP8_CLIP = 240.0          # ml_dtypes.float8_e4m3 max finite

LAMBDA_RECON, LAMBDA_RULE, LAMBDA_ATTN, LAMBDA_REG = 1.0, 0.5, 0.3, 1e-4
W_ATTN_GAT, W_ATTN_RULE = 0.5, 0.5

# BCE pad rows carry ms=0.5, rsb=0.5 -> d=ln1-ln2=0 so s1 gets nothing,
# s2 gets exactly ln(0.5) per pad row (subtracted on the host), and the
# strict rsb>0.5 violation test stays false.
_PAD_MS = 0.5

_NC = None


def _build_nc():
    """Build + compile the per-core Bass program (SPMD, identical on all
    cores; only the input shards differ)."""
    nc = bacc.Bacc("TRN2", target_bir_lowering=False, debug=False,
                   enable_asserts=True, num_devices=NCORES)

    pa = nc.dram_tensor("pa", [128, PA], FP8, kind="ExternalInput")
    pd = nc.dram_tensor("pd", [128, PD], FP8, kind="ExternalInput")
    ptab = nc.dram_tensor("ptab", [128, G * K], BF16, kind="ExternalInput")
    sc = nc.dram_tensor("sc", [128, 3 * G], F32, kind="ExternalInput")
    out = nc.dram_tensor("partials", [128, 16], F32, kind="ExternalOutput")

    a_off = np.cumsum([0] + A_TILES)
    d_off = np.cumsum([0] + D_TILES)

    with ExitStack() as ctx:
        tc = ctx.enter_context(tile.TileContext(nc))
        sp = ctx.enter_context(tc.tile_pool(name="small", bufs=1))
        pp = ctx.enter_context(tc.tile_pool(name="params", bufs=3))
        scrp = ctx.enter_context(tc.tile_pool(name="scratch", bufs=2))

        # ---- input DMAs spread over three rings so trigger serialization
        # overlaps and the compute engines' sequencers stay free: SP ring
        # carries sc + ACT tiles, the ACT ring only the table (one cheap
        # trigger before ACT's compute), SWDGE (gpsimd, idle) the DVE tiles.
        t_sc = sp.tile([128, 3 * G], F32)
        t_tab = sp.tile([128, G * K], BF16)
        pa_tiles = [None] * len(A_TILES)
        pd_tiles = [None] * len(D_TILES)

        nc.sync.dma_start(t_sc[:], sc.ap())
        nc.scalar.dma_start(t_tab[:], ptab.ap())
        for t in range(len(A_TILES)):
            tl = pp.tile([128, A_TILES[t]], FP8, tag=f"pa{t}")
            nc.sync.dma_start(tl[:], pa.ap()[:, int(a_off[t]):int(a_off[t + 1])])
            pa_tiles[t] = tl
        for t in range(len(D_TILES)):
            tl = pp.tile([128, D_TILES[t]], FP8, tag=f"pd{t}")
            nc.scalar.dma_start(tl[:], pd.ap()[:, int(d_off[t]):int(d_off[t + 1])])
            pd_tiles[t] = tl

        t_ms = t_sc[:, 0 * G:1 * G]
        t_rsb = t_sc[:, 1 * G:2 * G]
        t_bet = t_sc[:, 2 * G:3 * G]

        # parts: one column per partial quantity, written in place by
        # accum_out of the producing op; host folds the partition rows.
        #  0:s1  1:s2  2:srule  3:nv  4:sar  5:scnt  6:sgat  7:spa  8:spd
        parts = sp.tile([128, 16], F32)
        nc.vector.memset(parts[:], 0.0)

        # ---- scalar-engine ops: ln/bsq then the fp8 squares ----
        ln1 = sp.tile([128, G], F32)
        nc.scalar.activation(ln1[:], t_ms, ACTF.Ln)
        ln2 = sp.tile([128, G], F32)
        nc.scalar.activation(ln2[:], t_ms, ACTF.Ln, scale=-1.0, bias=1.0,
                             accum_out=parts[:, 1:2])
        bsq = sp.tile([128, G], F32)
        nc.scalar.activation(bsq[:], t_bet, ACTF.Square, scale=-1.0, bias=1.0)
        acca = sp.tile([128, len(A_TILES)], F32)
        for t in range(len(A_TILES)):
            sq = scrp.tile([128, A_TILES[t]], BF16, tag="sqa")
            nc.scalar.activation(sq[:], pa_tiles[t][:], ACTF.Square,
                                 accum_out=acca[:, t:t + 1])

        # ---- vector-engine ops ----
        # GAT rowmax via TT-max tree over the K=16 slots (bf16, 2x packed)
        tabv = t_tab[:].rearrange("p (k g) -> p k g", k=K)
        m8 = sp.tile([128, 8 * G], BF16)
        m8v = m8[:].rearrange("p (k g) -> p k g", k=8)
        nc.vector.tensor_tensor(m8v, tabv[:, 0:8, :], tabv[:, 8:16, :], ALU.max)
        m4 = sp.tile([128, 4 * G], BF16)
        m4v = m4[:].rearrange("p (k g) -> p k g", k=4)
        nc.vector.tensor_tensor(m4v, m8v[:, 0:4, :], m8v[:, 4:8, :], ALU.max)
        m2 = sp.tile([128, 2 * G], BF16)
        m2v = m2[:].rearrange("p (k g) -> p k g", k=2)
        nc.vector.tensor_tensor(m2v, m4v[:, 0:2, :], m4v[:, 2:4, :], ALU.max)
        rowmax = sp.tile([128, G], F32)
        nc.vector.tensor_tensor(rowmax[:], m2v[:, 0, :], m2v[:, 1, :], ALU.max)

        viol = sp.tile([128, G], F32)   # rsb>0.5 (strict: pad rows stay 0)
        nc.vector.tensor_scalar(viol[:], t_rsb, 0.5, 0.0,
                                ALU.is_gt, ALU.add, accum_out=parts[:, 3:4])
        has = sp.tile([128, G], F32)    # car has >=1 rule edge
        nc.vector.tensor_scalar(has[:], rowmax[:], 0.5, None, ALU.is_ge)
        dd = sp.tile([128, G], F32)     # 1 - payload == (1 - max_alpha)/2
        nc.vector.tensor_scalar(dd[:], rowmax[:], -1.0, 1.0, ALU.mult, ALU.add)

        gscr = sp.tile([128, G], F32)   # throwaway elementwise outputs
        d12 = sp.tile([128, G], F32)
        nc.vector.tensor_tensor(d12[:], ln1[:], ln2[:], ALU.subtract)
        nc.vector.scalar_tensor_tensor(gscr[:], t_rsb, 1.0, d12[:],
                                       ALU.mult, ALU.mult,
                                       accum_out=parts[:, 0:1])
        diff = sp.tile([128, G], F32)
        nc.vector.tensor_tensor(diff[:], t_ms, t_rsb, ALU.subtract)
        nc.vector.scalar_tensor_tensor(gscr[:], diff[:], 1.0, diff[:],
                                       ALU.mult, ALU.mult,
                                       accum_out=parts[:, 2:3])
        nc.vector.scalar_tensor_tensor(gscr[:], viol[:], 1.0, bsq[:],
                                       ALU.mult, ALU.mult,
                                       accum_out=parts[:, 4:5])
        valid = sp.tile([128, G], F32)
        nc.vector.scalar_tensor_tensor(valid[:], has[:], 1.0, viol[:],
                                       ALU.mult, ALU.mult,
                                       accum_out=parts[:, 5:6])
        vd = sp.tile([128, G], F32)     # valid*dd; then sum(vd*dd)=sum valid*dd^2
        nc.vector.scalar_tensor_tensor(vd[:], valid[:], 1.0, dd[:],
                                       ALU.mult, ALU.mult)
        nc.vector.scalar_tensor_tensor(gscr[:], vd[:], 1.0, dd[:],
                                       ALU.mult, ALU.mult,
                                       accum_out=parts[:, 6:7])

        # DVE param stream (fp8 STT sum-of-squares)
        accd = sp.tile([128, len(D_TILES)], F32)
        for t in range(len(D_TILES)):
            sq = scrp.tile([128, D_TILES[t]], BF16, tag="sqd")
            nc.vector.scalar_tensor_tensor(sq[:], pd_tiles[t][:], 1.0,
                                           pd_tiles[t][:], ALU.mult, ALU.mult,
                                           accum_out=accd[:, t:t + 1])

        nc.vector.tensor_reduce(parts[:, 7:8], acca[:],
                                mybir.AxisListType.X, ALU.add)
        nc.vector.tensor_reduce(parts[:, 8:9], accd[:],
                                mybir.AxisListType.X, ALU.add)

        # ---- ship per-partition partials; host folds the 128 rows ----
        nc.sync.dma_start(out.ap(), parts[:])

    nc.compile()
    return nc


def _get_nc():
    global _NC
    if _NC is None:
        _NC = _build_nc()
    return _NC


def prep_in_maps(inputs):
    """Host-side structure prep + sharding. Returns per-core input dicts."""
    ms = np.asarray(inputs["model_scores"], np.float32)
    rs = np.asarray(inputs["rule_scores"], np.float32)
    alpha = np.asarray(inputs["alpha_gat"], np.float32)
    beta = np.asarray(inputs["beta_rule"], np.float32)
    ei = np.asarray(inputs["edge_index"])
    et = np.asarray(inputs["entity_types"])
    p0 = np.asarray(inputs["param0"], np.float32)
    p1 = np.asarray(inputs["param1"], np.float32)

    src = ei[0].astype(np.int64, copy=False)
    dst = ei[1].astype(np.int64, copy=False)

    # rule edges: dst is a light (1) or stop line (2)
    rule_node = (et == 1) | (et == 2)
    sel = rule_node[dst]
    src_r = src[sel]
    a_r = alpha[sel]

    # group rule-edge alphas by source node (CSR-style)
    order = np.argsort(src_r, kind="stable")
    ssrc = src_r[order]
    sa = a_r[order]
    counts = np.bincount(ssrc, minlength=N)
    starts = np.zeros_like(counts)
    starts[1:] = np.cumsum(counts[:-1])

    # car ordinal -> node id (reference: nonzero(et==0, size=N_CAR), fill 0)
    car_ids = np.nonzero(et == 0)[0]
    if car_ids.size >= N_CAR:
        car_ids = car_ids[:N_CAR]
    else:
        car_ids = np.concatenate(
            [car_ids, np.zeros(N_CAR - car_ids.size, car_ids.dtype)])

    # padded [ROWS, K] table of (alpha+1)/2, one row per car ordinal
    cnt_full = counts[car_ids]
    cnt_ord = np.minimum(cnt_full, K)
    tot = int(cnt_ord.sum())
    row_idx = np.repeat(np.arange(N_CAR, dtype=np.int64), cnt_ord)
    cum = np.cumsum(cnt_ord) - cnt_ord
    within = np.arange(tot, dtype=np.int64) - np.repeat(cum, cnt_ord)
    srcpos = np.repeat(starts[car_ids], cnt_ord) + within
    ptab = np.zeros(ROWS * K, np.float32)
    ptab[row_idx * K + within] = (sa[srcpos] + np.float32(1.0)) * np.float32(0.5)
    # overflow fold (degree > K; not hit for the graded distribution)
    for i in np.nonzero(cnt_full > K)[0]:
        node = car_ids[i]
        extra = (sa[starts[node] + K:starts[node] + cnt_full[i]]
                 + np.float32(1.0)) * np.float32(0.5)
        ptab[i * K + K - 1] = max(ptab[i * K + K - 1], extra.max())
    ptab = ptab.reshape(ROWS, K)

    # padded score rows
    def pad(v, fill):
        o = np.full(ROWS, fill, np.float32)
        o[:N_CAR] = v
        return o

    ms_p = pad(ms, _PAD_MS)
    rsb_p = pad(rs, _PAD_MS)   # pad: ln(0.5) into s2 only; viol strict ✓
    bet_p = pad(beta, 1.0)

    # params: flatten, split per core, scale+convert to fp8
    pflat = np.concatenate([p0.reshape(-1), p1.reshape(-1)])
    pcore = pflat.reshape(NCORES, 128, PTOT)
    np_fp8 = mybir.dt.np(FP8)

    in_maps = []
    for c in range(NCORES):
        r0, r1 = c * RPC, (c + 1) * RPC
        scp = np.stack([ms_p[r0:r1].reshape(128, G),
                        rsb_p[r0:r1].reshape(128, G),
                        bet_p[r0:r1].reshape(128, G)], axis=1)
        tabc = (ptab[r0:r1].reshape(128, G, K).transpose(0, 2, 1)
                .astype(ml_dtypes.bfloat16))
        pblk = np.clip(pcore[c] * np.float32(FP8_SCALE),
                       -FP8_CLIP, FP8_CLIP).astype(np_fp8)
        in_maps.append({
            "pa": np.ascontiguousarray(pblk[:, :PA]),
            "pd": np.ascontiguousarray(pblk[:, PA:]),
            "ptab": np.ascontiguousarray(tabc).reshape(128, K * G),
            "sc": np.ascontiguousarray(scp).reshape(128, 3 * G),
        })
    return in_maps


def combine_partials(partials_per_core):
    """Host unshard: fold partition rows, add the 8 partial vectors,
    apply the scalar formula."""
    s = np.zeros(16, np.float64)
    for p in partials_per_core:
        s += np.asarray(p, np.float64).reshape(128, 16).sum(axis=0)
    s1, s2, s_rule, nv, s_ar, s_cnt, s_gat, s_pa, s_pd = s[:9]
    s_bce = s1 + s2 - NPAD * np.log(0.5)  # remove constant pad contribution

    L_recon = -s_bce / N_CAR
    L_rule = s_rule / N_CAR
    any_viol = nv > 0
    # payload = (alpha+1)/2 so (1-payload)^2 = (1-alpha)^2/4 -> x4 here
    L_attn_gat = (4.0 * s_gat / max(s_cnt, 1.0)) if (any_viol and s_cnt > 0) else 0.0
    L_attn_rule = (s_ar / max(nv, 1.0)) if any_viol else 0.0
    L_attn = W_ATTN_GAT * L_attn_gat + W_ATTN_RULE * L_attn_rule
    L_reg = (s_pa + s_pd) / (FP8_SCALE * FP8_SCALE)
    L_total = (LAMBDA_RECON * L_recon + LAMBDA_RULE * L_rule
               + LAMBDA_ATTN * L_attn + LAMBDA_REG * L_reg)
    return np.array([L_total, L_recon, L_rule, L_attn, L_attn_gat,
                     L_attn_rule, L_reg, nv], np.float32)


def kernel(**inputs):
    nc = _get_nc()
    in_maps = prep_in_maps(inputs)
    res = run_bass_kernel_spmd(nc, in_maps, list(range(NCORES)))
    return combine_partials([r["partials"] for r in res.results])
